# revision 11
# baseline (speedup 1.0000x reference)
"""MoE runtime-experts kernel for 8 Trainium2 NeuronCores.

Expert-parallel: core e holds expert e's weights. Host routes tokens by
expert id (argsort), pads each expert batch to a common capacity C, and
each core computes y = gelu(x @ W1 + b1) @ W2 + b2 for its batch as
dense matmuls in a transposed layout:

    L1: hT[hid, tok]  = W1[in, hid].T-contract  xT[in, tok]
    L2: yT[out, tok]  = W2[hid, out].T-contract hT[hid, tok]

Weights stay stationary on the PE (lhsT), tokens are the moving dim, so
activations flow through both layers without any on-device transpose.
Inputs/weights are cast to bf16 on host (PSUM accumulates fp32).

All streamed tensors (x, w1, w2, biases) are pre-swizzled on the host to
partition-major layouts so every DMA window is ONE contiguous run per
SBUF partition (128 descriptors). Naive [IN, HID]-style layouts cost
32*128 descriptors per window, and the Sync engine's DMA_DIRECT2D issue
blocks on descriptor generation (~4ns/desc) — the profile showed 17us(!)
single issue instructions starving both the DMA queues and the PE.
"""

import numpy as np
import ml_dtypes

import concourse.bass as bass
import concourse.mybir as mybir
import concourse.tile as tile
from concourse import bacc
from concourse.bass_utils import run_bass_kernel_spmd

P = 128
N_CORES = 8
BF16 = mybir.dt.bfloat16
F32 = mybir.dt.float32

_nc_cache = {}


def _tile_widths(C):
    """Near-equal tile widths (multiples of 4), as few tiles as SBUF
    allows (W<=376 keeps x/h/o pools under the ~209KB/partition cap,
    and W<=512 fits one PSUM bank per chain).

    Per-MM spacing measures W/2.4 + ~2.7ns for W <= 272, so equal
    widths near the cap amortize the fixed ~2.7ns and keep streaming
    (W/2.4ns) above the ~97ns LDWEIGHTS execute time (a narrow
    remainder tile would run at the LDWEIGHTS floor instead). W=348
    was measured MUCH slower (~176ns spacing, +21% vs W/2.4 — some
    per-MM cliff between W=272 and W=348), so the cap stays at 272."""
    n_t = max(1, -(-C // 272))
    base, rem = divmod(C // 4, n_t)
    return [4 * (base + (1 if i < rem else 0)) for i in range(n_t)]


def _ramp(total, first=1, cap=None):
    """Window sizes [1, 1, 2, 4, ...] (in m-tiles) doubling up to `cap`,
    summing to total. Small leading windows let dependent compute start
    early; big trailing windows amortize per-DMA issue overhead."""
    out, w, off = [], first, 0
    while off < total:
        take = min(w, total - off)
        out.append(take)
        off += take
        if len(out) >= 2:
            w *= 2
            if cap is not None:
                w = min(w, cap)
    return out


def _build_kernel(C, IN, HID, OUT, psum_bufs=8, repeat=1,
                  PIPE=2, GROUP=True, SPLIT_Y=True):
    K1, M1 = IN // P, HID // P
    K2, M2 = HID // P, OUT // P
    assert C % 16 == 0
    widths = _tile_widths(C)
    n_t = len(widths)
    starts_ = [sum(widths[:i]) for i in range(n_t)]
    xoffs = [K1 * s for s in starts_]
    # PSUM chains packed per 512-f32 bank: per-tile group size/stride.
    WMAX = max(widths)

    def _grp(W):
        gn = min(4, 512 // W)
        return gn, 512 // gn

    nc = bacc.Bacc("TRN2", target_bir_lowering=False, debug=False,
                   num_devices=N_CORES)
    # Swizzled layouts: tile t of xT is a [K1*W_t]-wide block per
    # partition, element (p, k*W_t+w) = x[token start_t+w, feature
    # k*128+p]; element (p, m, k*128+c) of w1 is w1[k*128+p, m*128+c];
    # likewise w2. Biases are [P, M] with (p, m) = b[m*128+p].
    xT = nc.dram_tensor("xT", [P, K1 * C], BF16, kind="ExternalInput")
    w1 = nc.dram_tensor("w1", [P, M1, K1 * P], BF16, kind="ExternalInput")
    w2 = nc.dram_tensor("w2", [P, M2, K2 * P], BF16, kind="ExternalInput")
    b1 = nc.dram_tensor("b1", [P, M1], F32, kind="ExternalInput")
    b2 = nc.dram_tensor("b2", [P, M2], F32, kind="ExternalInput")
    yT = nc.dram_tensor("yT", [OUT, C], F32, kind="ExternalOutput")

    with tile.TileContext(nc) as tc:
        with (
            tc.tile_pool(name="weights", bufs=1) as wpool,
            tc.tile_pool(name="xbuf",
                         bufs=(n_t if repeat == 1 else PIPE + 1)) as xpool,
            tc.tile_pool(name="hbuf", bufs=PIPE + 1) as hpool,
            tc.tile_pool(name="obuf",
                         bufs=(1 if WMAX > 200 else 2)) as opool,
            tc.tile_pool(name="psum", bufs=psum_bufs, space="PSUM") as pspool,
        ):
            w1_sb = wpool.tile([P, M1, K1 * P], BF16)
            w2_sb = wpool.tile([P, M2, K2 * P], BF16)
            b1_sb = wpool.tile([P, M1], F32)
            b2_sb = wpool.tile([P, M2], F32)

            depth = min(PIPE, n_t)
            # Single-shot: preload ALL x tiles into dedicated buffers (x is
            # only 2.3MB total). The first `depth` tiles go in front of the
            # weights so the L1 prefix can start immediately; the rest are
            # queued behind the weights (needed much later). DMA queue
            # order is issue order, so this sequencing is what keeps the
            # in-order Tensor stream fed.
            def _dma_x(x_sb, it, eng=None):
                nw = K1 * widths[it]
                (eng or nc.sync).dma_start(
                    x_sb[:, :nw], xT.ap()[:, xoffs[it]:xoffs[it] + nw])

            # Startup is DMA-ramp bound: queues deliver only ~100-180GB/s
            # for the first ~5us and each queue is FIFO, so the pieces the
            # first chain needs are spread across the THREE engine HWDGE
            # queues (sync/SP, scalar/Activation, gpsimd — the only ones
            # allowed to initiate DMAs) and issued in parallel (each DMA
            # issue costs ~650ns of its engine's time; engines issue
            # concurrently). First chain (m=0, k=0..7) gates on w1[m0]
            # (sync, two k-halves) and x0 (k-quarters alternating
            # scalar/gpsimd); biases follow on gpsimd. The rest of w1
            # ramps on sync in need order; every window is one contiguous
            # run per partition (128 descriptors).
            x_tiles = {}
            if repeat == 1:
                for it in range(n_t):
                    x_tiles[it] = xpool.tile([P, K1 * WMAX], BF16,
                                             tag="x", name="x_sb")
                W0 = widths[0]
                qk = max(K1 // 4, 1)
                qeng = (nc.scalar, nc.gpsimd)
                for i in range(0, K1, qk):
                    lo, hi = i * W0, min((i + qk) * W0, K1 * W0)
                    qeng[(i // qk) % 2].dma_start(x_tiles[0][:, lo:hi],
                                                  xT.ap()[:, lo:hi])
            hm = K1 * P // 2
            nc.sync.dma_start(w1_sb[:, :1, :hm], w1.ap()[:, :1, :hm])
            nc.sync.dma_start(w1_sb[:, :1, hm:], w1.ap()[:, :1, hm:])
            nc.gpsimd.dma_start(b1_sb[:], b1.ap())
            nc.gpsimd.dma_start(b2_sb[:], b2.ap())
            off = 1
            for w in _ramp(M1 - 1, first=1, cap=4):
                nc.sync.dma_start(w1_sb[:, off:off + w],
                                  w1.ap()[:, off:off + w])
                off += w
            if repeat == 1:
                # x1 is consumed by the depth-2 interleaved prefix almost
                # immediately, so it lands as two halves on both queues;
                # later tiles alternate scalar/gpsimd (slack-rich).
                if n_t > 1:
                    nw1, hx = K1 * widths[1], K1 * widths[1] // 2
                    o1 = xoffs[1]
                    nc.scalar.dma_start(x_tiles[1][:, :hx],
                                        xT.ap()[:, o1:o1 + hx])
                    nc.gpsimd.dma_start(x_tiles[1][:, hx:nw1],
                                        xT.ap()[:, o1 + hx:o1 + nw1])
                for it in range(2, n_t):
                    _dma_x(x_tiles[it], it,
                           eng=(nc.scalar if it % 2 == 0 else nc.gpsimd))
            off = 0
            for w in _ramp(M2, cap=2):
                nc.sync.dma_start(w2_sb[:, off:off + w],
                                  w2.ap()[:, off:off + w])
                off += w

            def l1_phase(it):
                x_sb = _get_x(it)
                W = widths[it]
                h_sb = hpool.tile([P, M1, WMAX], BF16, tag="h",
                                  name="h_sb")
                GN, SW = _grp(W)
                # Pack GN accumulation chains into one PSUM bank so the
                # slot-WAR sem wait is paid once per GN chains.
                for mg in range(0, M1, GN):
                    grp = range(mg, min(mg + GN, M1))
                    ps = pspool.tile([P, 512], F32, tag="ps", name="ps")
                    for mi, m in enumerate(grp):
                        for k in range(K1):
                            nc.tensor.matmul(
                                ps[:, mi * SW:mi * SW + W],
                                w1_sb[:, m, bass.ts(k, P)],
                                x_sb[:, k * W:(k + 1) * W],
                                start=(k == 0),
                                stop=(k == K1 - 1),
                            )
                    for mi, m in enumerate(grp):
                        nc.scalar.activation(
                            h_sb[:, m, :W],
                            ps[:, mi * SW:mi * SW + W],
                            mybir.ActivationFunctionType.Gelu,
                            bias=b1_sb[:, m:m + 1],
                        )
                return h_sb

            yTr = yT.ap().rearrange("(m p) c -> p m c", p=P)

            def l2_phase(it, h_sb, is_last=False):
                n0, W = starts_[it], widths[it]
                o_sb = opool.tile([P, M2, WMAX], F32, tag="o",
                                  name="o_sb")
                GN, SW = _grp(W)
                # Tail trim: on the last tile, each m-column's out-DMA is
                # issued right after its bias-add, round-robin over the
                # three DMA-capable queues, so the post-last-matmul drain
                # is one m-column's bias+issue+transfer instead of 8
                # transfers serialized on the sync queue (~3us on trace).
                oeng = (nc.sync, nc.scalar, nc.gpsimd)
                for mg in range(0, M2, GN):
                    grp = range(mg, min(mg + GN, M2))
                    ps = pspool.tile([P, 512], F32, tag="ps", name="ps")
                    for mi, m in enumerate(grp):
                        for k in range(K2):
                            nc.tensor.matmul(
                                ps[:, mi * SW:mi * SW + W],
                                w2_sb[:, m, bass.ts(k, P)],
                                h_sb[:, k, :W],
                                start=(k == 0),
                                stop=(k == K2 - 1),
                            )
                    for mi, m in enumerate(grp):
                        nc.vector.tensor_tensor(
                            o_sb[:, m, :W],
                            ps[:, mi * SW:mi * SW + W],
                            b2_sb[:, m:m + 1].to_broadcast((P, W)),
                            mybir.AluOpType.add,
                        )
                    if SPLIT_Y and is_last:
                        for mi, m in enumerate(grp):
                            oeng[m % 3].dma_start(yTr[:, m, n0:n0 + W],
                                                  o_sb[:, m, :W])
                if not (SPLIT_Y and is_last):
                    nc.sync.dma_start(yTr[:, :, n0:n0 + W], o_sb[:, :, :W])

            def _get_x(it):
                if it in x_tiles:
                    return x_tiles[it]
                x_sb = xpool.tile([P, K1 * WMAX], BF16, tag="x",
                                  name="x_sb")
                _dma_x(x_sb, it)
                return x_sb

            def l1_prefix(depth, hs):
                # m-interleaved L1 over the first `depth` tiles: one tile's
                # chains consume w1 m-columns at ~560GB/s, faster than the
                # ~430GB/s the DMA delivers w1 at startup. Spreading each
                # m-window over `depth` chains keeps the PE behind the DMA
                # so the weight load is fully hidden.
                xs = {it: _get_x(it) for it in range(depth)}
                for it in range(depth):
                    hs[it] = hpool.tile([P, M1, WMAX], BF16, tag="h",
                                        name="h_sb")
                GN, SW = _grp(max(widths[it] for it in range(depth)))
                chains = [(m, it) for m in range(M1) for it in range(depth)]
                for g in range(0, len(chains), GN):
                    grp = chains[g:g + GN]
                    ps = pspool.tile([P, 512], F32, tag="ps", name="ps")
                    for mi, (m, it) in enumerate(grp):
                        W = widths[it]
                        for k in range(K1):
                            nc.tensor.matmul(
                                ps[:, mi * SW:mi * SW + W],
                                w1_sb[:, m, bass.ts(k, P)],
                                xs[it][:, k * W:(k + 1) * W],
                                start=(k == 0),
                                stop=(k == K1 - 1),
                            )
                    for mi, (m, it) in enumerate(grp):
                        nc.scalar.activation(
                            hs[it][:, m, :widths[it]],
                            ps[:, mi * SW:mi * SW + widths[it]],
                            mybir.ActivationFunctionType.Gelu,
                            bias=b1_sb[:, m:m + 1],
                        )

            def body():
                # Software pipeline: L1 runs PIPE tiles ahead of L2 so the
                # w2 weight DMA tail hides behind L1 compute at startup.
                hs = {}
                l1_prefix(depth, hs)
                for j in range(n_t):
                    if j + depth < n_t:
                        hs[j + depth] = l1_phase(j + depth)
                    l2_phase(j, hs.pop(j), is_last=(j == n_t - 1))

            if repeat == 1:
                body()
            else:
                with tc.For_i(0, repeat, 1, name="rep"):
                    body()
    nc.compile()
    return nc


def _get_kernel(C, IN, HID, OUT):
    key = (C, IN, HID, OUT)
    if key not in _nc_cache:
        _nc_cache[key] = _build_kernel(C, IN, HID, OUT)
    return _nc_cache[key]


def prepare_in_maps(inputs):
    """Host-side routing: sort tokens by expert, pad to capacity C,
    build per-core swizzled input maps. Returns (in_maps, meta)."""
    x = np.ascontiguousarray(np.asarray(inputs["x"], dtype=np.float32))
    idx = np.asarray(inputs["indices_s"]).astype(np.int64)
    w1 = np.asarray(inputs["weight1"], dtype=np.float32)
    w2 = np.asarray(inputs["weight2"], dtype=np.float32)
    b1 = np.asarray(inputs["bias1"], dtype=np.float32)
    b2 = np.asarray(inputs["bias2"], dtype=np.float32)

    T = x.shape[0]
    E, IN, HID = w1.shape
    OUT = w2.shape[2]
    K1, M1 = IN // P, HID // P
    K2, M2 = HID // P, OUT // P
    assert E == N_CORES
    bf = ml_dtypes.bfloat16

    order = np.argsort(idx, kind="stable")
    counts = np.bincount(idx, minlength=E)
    starts = np.zeros(E + 1, dtype=np.int64)
    starts[1:] = np.cumsum(counts)
    # Capacity rounds to 16 (not 128); tiles split C near-equally.
    C = max(-(-int(counts.max()) // 16) * 16, 16)
    widths = _tile_widths(C)

    xbf = x.astype(bf)
    in_maps = []
    for e in range(E):
        toks = order[starts[e]:starts[e + 1]]
        xp = np.zeros((C, IN), dtype=bf)
        if len(toks):
            xp[:len(toks)] = xbf[toks]
        # per-tile blocks: (n0+w, k*128+p) -> [p, k*W+w], concatenated
        blocks = []
        n0 = 0
        for W in widths:
            blk = xp[n0:n0 + W].reshape(W, K1, P).transpose(2, 1, 0)
            blocks.append(blk.reshape(P, K1 * W))
            n0 += W
        x_sw = np.ascontiguousarray(np.concatenate(blocks, axis=1))
        # (k*128+p, m*128+c) -> [p, m, k*128+c]
        w1_sw = np.ascontiguousarray(
            w1[e].astype(bf).reshape(K1, P, M1, P).transpose(1, 2, 0, 3)
        ).reshape(P, M1, K1 * P)
        w2_sw = np.ascontiguousarray(
            w2[e].astype(bf).reshape(K2, P, M2, P).transpose(1, 2, 0, 3)
        ).reshape(P, M2, K2 * P)
        in_maps.append({
            "xT": x_sw,
            "w1": w1_sw,
            "w2": w2_sw,
            # biases pre-transposed to [P, M]
            "b1": np.ascontiguousarray(b1[e].reshape(M1, P).T),
            "b2": np.ascontiguousarray(b2[e].reshape(M2, P).T),
        })
    meta = {"key": (C, IN, HID, OUT), "order": order, "starts": starts,
            "T": T, "OUT": OUT}
    return in_maps, meta


def scatter_output(inputs, yT_all, meta):
    """Scatter per-core yT [E, OUT, C] back to [T, 1, OUT] fp32."""
    order, starts = meta["order"], meta["starts"]
    out = np.empty((meta["T"], meta["OUT"]), dtype=np.float32)
    for e in range(N_CORES):
        toks = order[starts[e]:starts[e + 1]]
        if len(toks):
            out[toks] = yT_all[e][:, :len(toks)].T
    return out[:, None, :]


def kernel(**inputs):
    in_maps, meta = prepare_in_maps(inputs)
    nc = _get_kernel(*meta["key"])
    res = run_bass_kernel_spmd(nc, in_maps, core_ids=list(range(N_CORES)),
                               trace=False)
    yT_all = np.stack([res.results[e]["yT"] for e in range(N_CORES)])
    return scatter_output(inputs, yT_all, meta)



# revision 14
# speedup vs baseline: 1.1636x; 1.1636x over previous
"""MoE runtime-experts kernel for 8 Trainium2 NeuronCores.

Expert-parallel: core e holds expert e's weights. Host routes tokens by
expert id (argsort), pads each expert batch to a common capacity C, and
each core computes y = gelu(x @ W1 + b1) @ W2 + b2 for its batch as
dense matmuls in a transposed layout:

    L1: hT[hid, tok]  = W1[in, hid].T-contract  xT[in, tok]
    L2: yT[out, tok]  = W2[hid, out].T-contract hT[hid, tok]

Weights stay stationary on the PE (lhsT), tokens are the moving dim, so
activations flow through both layers without any on-device transpose.
Inputs/weights are cast to bf16 on host (PSUM accumulates fp32).

All streamed tensors (x, w1, w2, biases) are pre-swizzled on the host to
partition-major layouts so every DMA window is ONE contiguous run per
SBUF partition (128 descriptors). Naive [IN, HID]-style layouts cost
32*128 descriptors per window, and the Sync engine's DMA_DIRECT2D issue
blocks on descriptor generation (~4ns/desc) — the profile showed 17us(!)
single issue instructions starving both the DMA queues and the PE.
"""

import numpy as np
import ml_dtypes

import concourse.bass as bass
import concourse.mybir as mybir
import concourse.tile as tile
from concourse import bacc
from concourse.bass_utils import run_bass_kernel_spmd

P = 128
N_CORES = 8
BF16 = mybir.dt.bfloat16
F32 = mybir.dt.float32

_nc_cache = {}


def _tile_widths(C):
    """Near-equal tile widths (multiples of 4), as few tiles as SBUF
    allows (W<=376 keeps x/h/o pools under the ~209KB/partition cap,
    and W<=512 fits one PSUM bank per chain).

    Widths are multiples of 8 so every per-k row stride (W*2 bytes)
    stays 16B-aligned in SBUF: misaligned strides slow the moving-
    operand stream ~20% (measured: W=256/272 clean at W/2.4+~3ns;
    W=260 -> 130ns, W=348 -> 176ns, both ~1.2x the aligned rate).
    Fewer, wider tiles amortize the fixed ~3ns/MM; W caps at 376 for
    SBUF (x/h/o pools) and 512 for one PSUM bank; W >= ~240 keeps the
    ~97ns LDWEIGHTS execute time hidden under the stream."""
    n_t = max(1, -(-C // 376))
    base, rem = divmod(C // 8, n_t)
    return [8 * (base + (1 if i < rem else 0)) for i in range(n_t)]


def _ramp(total, first=1, cap=None):
    """Window sizes [1, 1, 2, 4, ...] (in m-tiles) doubling up to `cap`,
    summing to total. Small leading windows let dependent compute start
    early; big trailing windows amortize per-DMA issue overhead."""
    out, w, off = [], first, 0
    while off < total:
        take = min(w, total - off)
        out.append(take)
        off += take
        if len(out) >= 2:
            w *= 2
            if cap is not None:
                w = min(w, cap)
    return out


def _build_kernel(C, IN, HID, OUT, psum_bufs=8, repeat=1,
                  PIPE=1, GROUP=True, SPLIT_Y=True):
    K1, M1 = IN // P, HID // P
    K2, M2 = HID // P, OUT // P
    assert C % 16 == 0
    widths = _tile_widths(C)
    n_t = len(widths)
    starts_ = [sum(widths[:i]) for i in range(n_t)]
    xoffs = [K1 * s for s in starts_]
    # PSUM chains packed per 512-f32 bank: per-tile group size/stride.
    WMAX = max(widths)

    def _grp(W):
        gn = min(4, 512 // W)
        return gn, 512 // gn

    nc = bacc.Bacc("TRN2", target_bir_lowering=False, debug=False,
                   num_devices=N_CORES)
    # Swizzled layouts: tile t of xT is a [K1*W_t]-wide block per
    # partition, element (p, k*W_t+w) = x[token start_t+w, feature
    # k*128+p]; element (p, m, k*128+c) of w1 is w1[k*128+p, m*128+c];
    # likewise w2. Biases are [P, M] with (p, m) = b[m*128+p].
    xT = nc.dram_tensor("xT", [P, K1 * C], BF16, kind="ExternalInput")
    w1 = nc.dram_tensor("w1", [P, M1, K1 * P], BF16, kind="ExternalInput")
    w2 = nc.dram_tensor("w2", [P, M2, K2 * P], BF16, kind="ExternalInput")
    b1 = nc.dram_tensor("b1", [P, M1], F32, kind="ExternalInput")
    b2 = nc.dram_tensor("b2", [P, M2], F32, kind="ExternalInput")
    yT = nc.dram_tensor("yT", [OUT, C], F32, kind="ExternalOutput")

    with tile.TileContext(nc) as tc:
        with (
            tc.tile_pool(name="weights", bufs=1) as wpool,
            tc.tile_pool(name="xbuf",
                         bufs=(n_t if repeat == 1 else PIPE + 1)) as xpool,
            tc.tile_pool(name="hbuf", bufs=PIPE + 1) as hpool,
            tc.tile_pool(name="obuf",
                         bufs=(1 if WMAX > 200 else 2)) as opool,
            tc.tile_pool(name="psum", bufs=psum_bufs, space="PSUM") as pspool,
        ):
            w1_sb = wpool.tile([P, M1, K1 * P], BF16)
            w2_sb = wpool.tile([P, M2, K2 * P], BF16)
            b1_sb = wpool.tile([P, M1], F32)
            b2_sb = wpool.tile([P, M2], F32)

            depth = min(PIPE, n_t)
            # Single-shot: preload ALL x tiles into dedicated buffers (x is
            # only 2.3MB total). The first `depth` tiles go in front of the
            # weights so the L1 prefix can start immediately; the rest are
            # queued behind the weights (needed much later). DMA queue
            # order is issue order, so this sequencing is what keeps the
            # in-order Tensor stream fed.
            def _dma_x(x_sb, it, eng=None):
                nw = K1 * widths[it]
                (eng or nc.sync).dma_start(
                    x_sb[:, :nw], xT.ap()[:, xoffs[it]:xoffs[it] + nw])

            # Startup is DMA-ramp bound: queues deliver only ~100-180GB/s
            # for the first ~5us and each queue is FIFO, so the pieces the
            # first chain needs are spread across the THREE engine HWDGE
            # queues (sync/SP, scalar/Activation, gpsimd — the only ones
            # allowed to initiate DMAs) and issued in parallel (each DMA
            # issue costs ~650ns of its engine's time; engines issue
            # concurrently). First chain (m=0, k=0..7) gates on w1[m0]
            # (sync, two k-halves) and x0 (k-quarters alternating
            # scalar/gpsimd); biases follow on gpsimd. The rest of w1
            # ramps on sync in need order; every window is one contiguous
            # run per partition (128 descriptors).
            x_tiles = {}
            if repeat == 1:
                for it in range(n_t):
                    x_tiles[it] = xpool.tile([P, K1 * WMAX], BF16,
                                             tag="x", name="x_sb")
                W0 = widths[0]
                qk = max(K1 // 4, 1)
                qeng = (nc.scalar, nc.gpsimd)
                for i in range(0, K1, qk):
                    lo, hi = i * W0, min((i + qk) * W0, K1 * W0)
                    qeng[(i // qk) % 2].dma_start(x_tiles[0][:, lo:hi],
                                                  xT.ap()[:, lo:hi])
            # w1's first four m-columns land from three queues in
            # parallel (m0 halves on sync, m1/m2 behind the x quarters on
            # scalar, m3 on gpsimd) so the first ~5 chains never starve
            # while the per-queue DMA rate ramps; sync then carries the
            # rest of w1 in need order.
            hm = K1 * P // 2
            nc.sync.dma_start(w1_sb[:, :1, :hm], w1.ap()[:, :1, :hm])
            nc.sync.dma_start(w1_sb[:, :1, hm:], w1.ap()[:, :1, hm:])
            w1_done = 1
            if repeat == 1 and M1 >= 4:
                nc.scalar.dma_start(w1_sb[:, 1:2], w1.ap()[:, 1:2])
                nc.scalar.dma_start(w1_sb[:, 2:3], w1.ap()[:, 2:3])
                nc.gpsimd.dma_start(w1_sb[:, 3:4], w1.ap()[:, 3:4])
                w1_done = 4
            nc.gpsimd.dma_start(b1_sb[:], b1.ap())
            nc.gpsimd.dma_start(b2_sb[:], b2.ap())
            off = w1_done
            for w in _ramp(M1 - w1_done, first=4, cap=4):
                nc.sync.dma_start(w1_sb[:, off:off + w],
                                  w1.ap()[:, off:off + w])
                off += w
            if repeat == 1:
                # later x tiles have tens of us of slack; alternate the
                # two non-critical queues
                for it in range(1, n_t):
                    _dma_x(x_tiles[it], it,
                           eng=(nc.scalar if it % 2 == 1 else nc.gpsimd))
            off = 0
            for w in _ramp(M2, cap=2):
                nc.sync.dma_start(w2_sb[:, off:off + w],
                                  w2.ap()[:, off:off + w])
                off += w

            def l1_phase(it):
                x_sb = _get_x(it)
                W = widths[it]
                h_sb = hpool.tile([P, M1, WMAX], BF16, tag="h",
                                  name="h_sb")
                GN, SW = _grp(W)
                # Pack GN accumulation chains into one PSUM bank so the
                # slot-WAR sem wait is paid once per GN chains.
                for mg in range(0, M1, GN):
                    grp = range(mg, min(mg + GN, M1))
                    ps = pspool.tile([P, 512], F32, tag="ps", name="ps")
                    for mi, m in enumerate(grp):
                        for k in range(K1):
                            nc.tensor.matmul(
                                ps[:, mi * SW:mi * SW + W],
                                w1_sb[:, m, bass.ts(k, P)],
                                x_sb[:, k * W:(k + 1) * W],
                                start=(k == 0),
                                stop=(k == K1 - 1),
                            )
                    for mi, m in enumerate(grp):
                        nc.scalar.activation(
                            h_sb[:, m, :W],
                            ps[:, mi * SW:mi * SW + W],
                            mybir.ActivationFunctionType.Gelu,
                            bias=b1_sb[:, m:m + 1],
                        )
                return h_sb

            yTr = yT.ap().rearrange("(m p) c -> p m c", p=P)

            def l2_phase(it, h_sb, is_last=False):
                n0, W = starts_[it], widths[it]
                o_sb = opool.tile([P, M2, WMAX], F32, tag="o",
                                  name="o_sb")
                GN, SW = _grp(W)
                # Tail trim: on the last tile, each m-column's out-DMA is
                # issued right after its bias-add, round-robin over the
                # three DMA-capable queues, so the post-last-matmul drain
                # is one m-column's bias+issue+transfer instead of 8
                # transfers serialized on the sync queue (~3us on trace).
                oeng = (nc.sync, nc.scalar, nc.gpsimd)
                for mg in range(0, M2, GN):
                    grp = range(mg, min(mg + GN, M2))
                    ps = pspool.tile([P, 512], F32, tag="ps", name="ps")
                    for mi, m in enumerate(grp):
                        for k in range(K2):
                            nc.tensor.matmul(
                                ps[:, mi * SW:mi * SW + W],
                                w2_sb[:, m, bass.ts(k, P)],
                                h_sb[:, k, :W],
                                start=(k == 0),
                                stop=(k == K2 - 1),
                            )
                    for mi, m in enumerate(grp):
                        nc.vector.tensor_tensor(
                            o_sb[:, m, :W],
                            ps[:, mi * SW:mi * SW + W],
                            b2_sb[:, m:m + 1].to_broadcast((P, W)),
                            mybir.AluOpType.add,
                        )
                    if SPLIT_Y and is_last:
                        for mi, m in enumerate(grp):
                            oeng[m % 3].dma_start(yTr[:, m, n0:n0 + W],
                                                  o_sb[:, m, :W])
                if not (SPLIT_Y and is_last):
                    nc.sync.dma_start(yTr[:, :, n0:n0 + W], o_sb[:, :, :W])

            def _get_x(it):
                if it in x_tiles:
                    return x_tiles[it]
                x_sb = xpool.tile([P, K1 * WMAX], BF16, tag="x",
                                  name="x_sb")
                _dma_x(x_sb, it)
                return x_sb

            def l1_prefix(depth, hs):
                # m-interleaved L1 over the first `depth` tiles: one tile's
                # chains consume w1 m-columns at ~560GB/s, faster than the
                # ~430GB/s the DMA delivers w1 at startup. Spreading each
                # m-window over `depth` chains keeps the PE behind the DMA
                # so the weight load is fully hidden.
                xs = {it: _get_x(it) for it in range(depth)}
                for it in range(depth):
                    hs[it] = hpool.tile([P, M1, WMAX], BF16, tag="h",
                                        name="h_sb")
                GN, SW = _grp(max(widths[it] for it in range(depth)))
                chains = [(m, it) for m in range(M1) for it in range(depth)]
                for g in range(0, len(chains), GN):
                    grp = chains[g:g + GN]
                    ps = pspool.tile([P, 512], F32, tag="ps", name="ps")
                    for mi, (m, it) in enumerate(grp):
                        W = widths[it]
                        for k in range(K1):
                            nc.tensor.matmul(
                                ps[:, mi * SW:mi * SW + W],
                                w1_sb[:, m, bass.ts(k, P)],
                                xs[it][:, k * W:(k + 1) * W],
                                start=(k == 0),
                                stop=(k == K1 - 1),
                            )
                    for mi, (m, it) in enumerate(grp):
                        nc.scalar.activation(
                            hs[it][:, m, :widths[it]],
                            ps[:, mi * SW:mi * SW + widths[it]],
                            mybir.ActivationFunctionType.Gelu,
                            bias=b1_sb[:, m:m + 1],
                        )

            def body():
                # Software pipeline: L1 runs PIPE tiles ahead of L2 so the
                # w2 weight DMA tail hides behind L1 compute at startup.
                hs = {}
                l1_prefix(depth, hs)
                for j in range(n_t):
                    if j + depth < n_t:
                        hs[j + depth] = l1_phase(j + depth)
                    l2_phase(j, hs.pop(j), is_last=(j == n_t - 1))

            if repeat == 1:
                body()
            else:
                with tc.For_i(0, repeat, 1, name="rep"):
                    body()
    nc.compile()
    return nc


def _get_kernel(C, IN, HID, OUT):
    key = (C, IN, HID, OUT)
    if key not in _nc_cache:
        _nc_cache[key] = _build_kernel(C, IN, HID, OUT)
    return _nc_cache[key]


def prepare_in_maps(inputs):
    """Host-side routing: sort tokens by expert, pad to capacity C,
    build per-core swizzled input maps. Returns (in_maps, meta)."""
    x = np.ascontiguousarray(np.asarray(inputs["x"], dtype=np.float32))
    idx = np.asarray(inputs["indices_s"]).astype(np.int64)
    w1 = np.asarray(inputs["weight1"], dtype=np.float32)
    w2 = np.asarray(inputs["weight2"], dtype=np.float32)
    b1 = np.asarray(inputs["bias1"], dtype=np.float32)
    b2 = np.asarray(inputs["bias2"], dtype=np.float32)

    T = x.shape[0]
    E, IN, HID = w1.shape
    OUT = w2.shape[2]
    K1, M1 = IN // P, HID // P
    K2, M2 = HID // P, OUT // P
    assert E == N_CORES
    bf = ml_dtypes.bfloat16

    order = np.argsort(idx, kind="stable")
    counts = np.bincount(idx, minlength=E)
    starts = np.zeros(E + 1, dtype=np.int64)
    starts[1:] = np.cumsum(counts)
    # Capacity rounds to 16 (not 128); tiles split C near-equally.
    C = max(-(-int(counts.max()) // 16) * 16, 16)
    widths = _tile_widths(C)

    xbf = x.astype(bf)
    in_maps = []
    for e in range(E):
        toks = order[starts[e]:starts[e + 1]]
        xp = np.zeros((C, IN), dtype=bf)
        if len(toks):
            xp[:len(toks)] = xbf[toks]
        # per-tile blocks: (n0+w, k*128+p) -> [p, k*W+w], concatenated
        blocks = []
        n0 = 0
        for W in widths:
            blk = xp[n0:n0 + W].reshape(W, K1, P).transpose(2, 1, 0)
            blocks.append(blk.reshape(P, K1 * W))
            n0 += W
        x_sw = np.ascontiguousarray(np.concatenate(blocks, axis=1))
        # (k*128+p, m*128+c) -> [p, m, k*128+c]
        w1_sw = np.ascontiguousarray(
            w1[e].astype(bf).reshape(K1, P, M1, P).transpose(1, 2, 0, 3)
        ).reshape(P, M1, K1 * P)
        w2_sw = np.ascontiguousarray(
            w2[e].astype(bf).reshape(K2, P, M2, P).transpose(1, 2, 0, 3)
        ).reshape(P, M2, K2 * P)
        in_maps.append({
            "xT": x_sw,
            "w1": w1_sw,
            "w2": w2_sw,
            # biases pre-transposed to [P, M]
            "b1": np.ascontiguousarray(b1[e].reshape(M1, P).T),
            "b2": np.ascontiguousarray(b2[e].reshape(M2, P).T),
        })
    meta = {"key": (C, IN, HID, OUT), "order": order, "starts": starts,
            "T": T, "OUT": OUT}
    return in_maps, meta


def scatter_output(inputs, yT_all, meta):
    """Scatter per-core yT [E, OUT, C] back to [T, 1, OUT] fp32."""
    order, starts = meta["order"], meta["starts"]
    out = np.empty((meta["T"], meta["OUT"]), dtype=np.float32)
    for e in range(N_CORES):
        toks = order[starts[e]:starts[e + 1]]
        if len(toks):
            out[toks] = yT_all[e][:, :len(toks)].T
    return out[:, None, :]


def kernel(**inputs):
    in_maps, meta = prepare_in_maps(inputs)
    nc = _get_kernel(*meta["key"])
    res = run_bass_kernel_spmd(nc, in_maps, core_ids=list(range(N_CORES)),
                               trace=False)
    yT_all = np.stack([res.results[e]["yT"] for e in range(N_CORES)])
    return scatter_output(inputs, yT_all, meta)



# revision 17
# speedup vs baseline: 1.1681x; 1.0039x over previous
"""MoE runtime-experts kernel for 8 Trainium2 NeuronCores.

Expert-parallel: core e holds expert e's weights. Host routes tokens by
expert id (argsort), pads each expert batch to a common capacity C, and
each core computes y = gelu(x @ W1 + b1) @ W2 + b2 for its batch as
dense matmuls in a transposed layout:

    L1: hT[hid, tok]  = W1[in, hid].T-contract  xT[in, tok]
    L2: yT[out, tok]  = W2[hid, out].T-contract hT[hid, tok]

Weights stay stationary on the PE (lhsT), tokens are the moving dim, so
activations flow through both layers without any on-device transpose.
Inputs/weights are cast to bf16 on host (PSUM accumulates fp32).

All streamed tensors (x, w1, w2, biases) are pre-swizzled on the host to
partition-major layouts so every DMA window is ONE contiguous run per
SBUF partition (128 descriptors). Naive [IN, HID]-style layouts cost
32*128 descriptors per window, and the Sync engine's DMA_DIRECT2D issue
blocks on descriptor generation (~4ns/desc) — the profile showed 17us(!)
single issue instructions starving both the DMA queues and the PE.
"""

import numpy as np
import ml_dtypes

import concourse.bass as bass
import concourse.mybir as mybir
import concourse.tile as tile
from concourse import bacc
from concourse.bass_utils import run_bass_kernel_spmd

P = 128
N_CORES = 8
BF16 = mybir.dt.bfloat16
F32 = mybir.dt.float32

_nc_cache = {}


def _tile_widths(C):
    """Near-equal tile widths (multiples of 4), as few tiles as SBUF
    allows (W<=376 keeps x/h/o pools under the ~209KB/partition cap,
    and W<=512 fits one PSUM bank per chain).

    Widths are multiples of 8 so every per-k row stride (W*2 bytes)
    stays 16B-aligned in SBUF: misaligned strides slow the moving-
    operand stream ~20% (measured: W=256/272 clean at W/2.4+~3ns;
    W=260 -> 130ns, W=348 -> 176ns, both ~1.2x the aligned rate).
    Wider tiles amortize the fixed ~3ns/MM, but W caps at 264: the
    PIPE=2 interleaved prefix (which halves the startup w1 bandwidth
    demand — PIPE=1 at W~350 measured 12.5us of early PE starvation)
    needs 3 h-buffers, and SBUF fits 131KB weights + 288*W of pools
    only for W<=271. W >= ~240 keeps the ~97ns LDWEIGHTS execute time
    hidden under the stream. Widest tiles go first (lowest prefix w1
    demand; smallest last tile shortens the drain tail)."""
    n_t = max(1, -(-C // 264))
    base, rem = divmod(C // 8, n_t)
    return [8 * (base + (1 if i < rem else 0)) for i in range(n_t)]


def _ramp(total, first=1, cap=None):
    """Window sizes [1, 1, 2, 4, ...] (in m-tiles) doubling up to `cap`,
    summing to total. Small leading windows let dependent compute start
    early; big trailing windows amortize per-DMA issue overhead."""
    out, w, off = [], first, 0
    while off < total:
        take = min(w, total - off)
        out.append(take)
        off += take
        if len(out) >= 2:
            w *= 2
            if cap is not None:
                w = min(w, cap)
    return out


def _build_kernel(C, IN, HID, OUT, psum_bufs=8, repeat=1,
                  PIPE=2, GROUP=True, SPLIT_Y=True):
    K1, M1 = IN // P, HID // P
    K2, M2 = HID // P, OUT // P
    assert C % 16 == 0
    widths = _tile_widths(C)
    n_t = len(widths)
    starts_ = [sum(widths[:i]) for i in range(n_t)]
    xoffs = [K1 * s for s in starts_]
    # PSUM chains packed per 512-f32 bank: per-tile group size/stride.
    WMAX = max(widths)

    def _grp(W):
        gn = min(4, 512 // W)
        return gn, 512 // gn

    nc = bacc.Bacc("TRN2", target_bir_lowering=False, debug=False,
                   num_devices=N_CORES)
    # Swizzled layouts: tile t of xT is a [K1*W_t]-wide block per
    # partition, element (p, k*W_t+w) = x[token start_t+w, feature
    # k*128+p]; element (p, m, k*128+c) of w1 is w1[k*128+p, m*128+c];
    # likewise w2. Biases are [P, M] with (p, m) = b[m*128+p].
    xT = nc.dram_tensor("xT", [P, K1 * C], BF16, kind="ExternalInput")
    w1 = nc.dram_tensor("w1", [P, M1, K1 * P], BF16, kind="ExternalInput")
    w2 = nc.dram_tensor("w2", [P, M2, K2 * P], BF16, kind="ExternalInput")
    b1 = nc.dram_tensor("b1", [P, M1], F32, kind="ExternalInput")
    b2 = nc.dram_tensor("b2", [P, M2], F32, kind="ExternalInput")
    yT = nc.dram_tensor("yT", [OUT, C], F32, kind="ExternalOutput")

    with tile.TileContext(nc) as tc:
        with (
            tc.tile_pool(name="weights", bufs=1) as wpool,
            tc.tile_pool(name="xbuf",
                         bufs=(n_t if repeat == 1 else PIPE + 1)) as xpool,
            tc.tile_pool(name="hbuf", bufs=PIPE + 1) as hpool,
            tc.tile_pool(name="obuf",
                         bufs=(1 if WMAX > 200 else 2)) as opool,
            tc.tile_pool(name="psum", bufs=psum_bufs, space="PSUM") as pspool,
        ):
            w1_sb = wpool.tile([P, M1, K1 * P], BF16)
            w2_sb = wpool.tile([P, M2, K2 * P], BF16)
            b1_sb = wpool.tile([P, M1], F32)
            b2_sb = wpool.tile([P, M2], F32)

            depth = min(PIPE, n_t)
            # Single-shot: preload ALL x tiles into dedicated buffers (x is
            # only 2.3MB total). The first `depth` tiles go in front of the
            # weights so the L1 prefix can start immediately; the rest are
            # queued behind the weights (needed much later). DMA queue
            # order is issue order, so this sequencing is what keeps the
            # in-order Tensor stream fed.
            def _dma_x(x_sb, it, eng=None):
                nw = K1 * widths[it]
                (eng or nc.sync).dma_start(
                    x_sb[:, :nw], xT.ap()[:, xoffs[it]:xoffs[it] + nw])

            # Startup is DMA-ramp bound: queues deliver only ~100-180GB/s
            # for the first ~5us and each queue is FIFO, so the pieces the
            # first chain needs are spread across the THREE engine HWDGE
            # queues (sync/SP, scalar/Activation, gpsimd — the only ones
            # allowed to initiate DMAs) and issued in parallel (each DMA
            # issue costs ~650ns of its engine's time; engines issue
            # concurrently). First chain (m=0, k=0..7) gates on w1[m0]
            # (sync, two k-halves) and x0 (k-quarters alternating
            # scalar/gpsimd); biases follow on gpsimd. The rest of w1
            # ramps on sync in need order; every window is one contiguous
            # run per partition (128 descriptors).
            x_tiles = {}

            def _x_quarters(it, eng):
                W = widths[it]
                qk = max(K1 // 4, 1)
                for i in range(0, K1, qk):
                    lo, hi = i * W, min((i + qk) * W, K1 * W)
                    eng.dma_start(x_tiles[it][:, lo:hi],
                                  xT.ap()[:, xoffs[it] + lo:xoffs[it] + hi])

            if repeat == 1:
                for it in range(n_t):
                    x_tiles[it] = xpool.tile([P, K1 * WMAX], BF16,
                                             tag="x", name="x_sb")
                # scalar carries x0 (k-quarters, so matmul k can start as
                # soon as its quarter lands); gpsimd carries x1 the same
                # way (the depth-2 prefix consumes x1 ~0.9us after x0),
                # with b1 slotted after the first quarter.
                _x_quarters(0, nc.scalar)
                if n_t > 1:
                    qk = max(K1 // 4, 1)
                    W1t = widths[1]
                    nc.gpsimd.dma_start(
                        x_tiles[1][:, :qk * W1t],
                        xT.ap()[:, xoffs[1]:xoffs[1] + qk * W1t])
                    nc.gpsimd.dma_start(b1_sb[:], b1.ap())
                    for i in range(qk, K1, qk):
                        lo, hi = i * W1t, min((i + qk) * W1t, K1 * W1t)
                        nc.gpsimd.dma_start(
                            x_tiles[1][:, lo:hi],
                            xT.ap()[:, xoffs[1] + lo:xoffs[1] + hi])
                else:
                    nc.gpsimd.dma_start(b1_sb[:], b1.ap())
            else:
                nc.gpsimd.dma_start(b1_sb[:], b1.ap())
            # Early w1 m-columns come from all three queues in parallel
            # (sync: m0 halves then m1; scalar m2/m4 behind x0; gpsimd
            # m3 behind x1) so the prefix chains never starve while the
            # per-queue DMA rate ramps from ~100 to ~380GB/s over the
            # first ~8us; sync then carries the rest in need order.
            hm = K1 * P // 2
            nc.sync.dma_start(w1_sb[:, :1, :hm], w1.ap()[:, :1, :hm])
            nc.sync.dma_start(w1_sb[:, :1, hm:], w1.ap()[:, :1, hm:])
            w1_done = 1
            if repeat == 1 and M1 >= 8:
                nc.sync.dma_start(w1_sb[:, 1:2], w1.ap()[:, 1:2])
                nc.scalar.dma_start(w1_sb[:, 2:3], w1.ap()[:, 2:3])
                nc.gpsimd.dma_start(w1_sb[:, 3:4], w1.ap()[:, 3:4])
                nc.scalar.dma_start(w1_sb[:, 4:5], w1.ap()[:, 4:5])
                w1_done = 5
            nc.gpsimd.dma_start(b2_sb[:], b2.ap())
            off = w1_done
            for w in _ramp(M1 - w1_done, first=1, cap=4):
                nc.sync.dma_start(w1_sb[:, off:off + w],
                                  w1.ap()[:, off:off + w])
                off += w
            if repeat == 1:
                # later x tiles have tens of us of slack; alternate the
                # two non-critical queues
                for it in range(2, n_t):
                    _dma_x(x_tiles[it], it,
                           eng=(nc.scalar if it % 2 == 0 else nc.gpsimd))
            off = 0
            for w in _ramp(M2, cap=2):
                nc.sync.dma_start(w2_sb[:, off:off + w],
                                  w2.ap()[:, off:off + w])
                off += w

            def l1_phase(it):
                x_sb = _get_x(it)
                W = widths[it]
                h_sb = hpool.tile([P, M1, WMAX], BF16, tag="h",
                                  name="h_sb")
                GN, SW = _grp(W)
                # Pack GN accumulation chains into one PSUM bank so the
                # slot-WAR sem wait is paid once per GN chains.
                for mg in range(0, M1, GN):
                    grp = range(mg, min(mg + GN, M1))
                    ps = pspool.tile([P, 512], F32, tag="ps", name="ps")
                    for mi, m in enumerate(grp):
                        for k in range(K1):
                            nc.tensor.matmul(
                                ps[:, mi * SW:mi * SW + W],
                                w1_sb[:, m, bass.ts(k, P)],
                                x_sb[:, k * W:(k + 1) * W],
                                start=(k == 0),
                                stop=(k == K1 - 1),
                            )
                    for mi, m in enumerate(grp):
                        nc.scalar.activation(
                            h_sb[:, m, :W],
                            ps[:, mi * SW:mi * SW + W],
                            mybir.ActivationFunctionType.Gelu,
                            bias=b1_sb[:, m:m + 1],
                        )
                return h_sb

            yTr = yT.ap().rearrange("(m p) c -> p m c", p=P)

            def l2_phase(it, h_sb, is_last=False):
                n0, W = starts_[it], widths[it]
                o_sb = opool.tile([P, M2, WMAX], F32, tag="o",
                                  name="o_sb")
                GN, SW = _grp(W)
                # Tail trim: on the last tile, each m-column's out-DMA is
                # issued right after its bias-add, round-robin over the
                # three DMA-capable queues, so the post-last-matmul drain
                # is one m-column's bias+issue+transfer instead of 8
                # transfers serialized on the sync queue (~3us on trace).
                oeng = (nc.sync, nc.scalar, nc.gpsimd)
                for mg in range(0, M2, GN):
                    grp = range(mg, min(mg + GN, M2))
                    ps = pspool.tile([P, 512], F32, tag="ps", name="ps")
                    for mi, m in enumerate(grp):
                        for k in range(K2):
                            nc.tensor.matmul(
                                ps[:, mi * SW:mi * SW + W],
                                w2_sb[:, m, bass.ts(k, P)],
                                h_sb[:, k, :W],
                                start=(k == 0),
                                stop=(k == K2 - 1),
                            )
                    for mi, m in enumerate(grp):
                        nc.vector.tensor_tensor(
                            o_sb[:, m, :W],
                            ps[:, mi * SW:mi * SW + W],
                            b2_sb[:, m:m + 1].to_broadcast((P, W)),
                            mybir.AluOpType.add,
                        )
                    if SPLIT_Y and is_last:
                        for mi, m in enumerate(grp):
                            oeng[m % 3].dma_start(yTr[:, m, n0:n0 + W],
                                                  o_sb[:, m, :W])
                if not (SPLIT_Y and is_last):
                    nc.sync.dma_start(yTr[:, :, n0:n0 + W], o_sb[:, :, :W])

            def _get_x(it):
                if it in x_tiles:
                    return x_tiles[it]
                x_sb = xpool.tile([P, K1 * WMAX], BF16, tag="x",
                                  name="x_sb")
                _dma_x(x_sb, it)
                return x_sb

            def l1_prefix(depth, hs):
                # m-interleaved L1 over the first `depth` tiles: one tile's
                # chains consume w1 m-columns at ~560GB/s, faster than the
                # ~430GB/s the DMA delivers w1 at startup. Spreading each
                # m-window over `depth` chains keeps the PE behind the DMA
                # so the weight load is fully hidden.
                xs = {it: _get_x(it) for it in range(depth)}
                for it in range(depth):
                    hs[it] = hpool.tile([P, M1, WMAX], BF16, tag="h",
                                        name="h_sb")
                GN, SW = _grp(max(widths[it] for it in range(depth)))
                chains = [(m, it) for m in range(M1) for it in range(depth)]
                for g in range(0, len(chains), GN):
                    grp = chains[g:g + GN]
                    ps = pspool.tile([P, 512], F32, tag="ps", name="ps")
                    for mi, (m, it) in enumerate(grp):
                        W = widths[it]
                        for k in range(K1):
                            nc.tensor.matmul(
                                ps[:, mi * SW:mi * SW + W],
                                w1_sb[:, m, bass.ts(k, P)],
                                xs[it][:, k * W:(k + 1) * W],
                                start=(k == 0),
                                stop=(k == K1 - 1),
                            )
                    for mi, (m, it) in enumerate(grp):
                        nc.scalar.activation(
                            hs[it][:, m, :widths[it]],
                            ps[:, mi * SW:mi * SW + widths[it]],
                            mybir.ActivationFunctionType.Gelu,
                            bias=b1_sb[:, m:m + 1],
                        )

            def body():
                # Software pipeline: L1 runs PIPE tiles ahead of L2 so the
                # w2 weight DMA tail hides behind L1 compute at startup.
                hs = {}
                l1_prefix(depth, hs)
                for j in range(n_t):
                    if j + depth < n_t:
                        hs[j + depth] = l1_phase(j + depth)
                    l2_phase(j, hs.pop(j), is_last=(j == n_t - 1))

            if repeat == 1:
                body()
            else:
                with tc.For_i(0, repeat, 1, name="rep"):
                    body()
    nc.compile()
    return nc


def _get_kernel(C, IN, HID, OUT):
    key = (C, IN, HID, OUT)
    if key not in _nc_cache:
        _nc_cache[key] = _build_kernel(C, IN, HID, OUT)
    return _nc_cache[key]


def prepare_in_maps(inputs):
    """Host-side routing: sort tokens by expert, pad to capacity C,
    build per-core swizzled input maps. Returns (in_maps, meta)."""
    x = np.ascontiguousarray(np.asarray(inputs["x"], dtype=np.float32))
    idx = np.asarray(inputs["indices_s"]).astype(np.int64)
    w1 = np.asarray(inputs["weight1"], dtype=np.float32)
    w2 = np.asarray(inputs["weight2"], dtype=np.float32)
    b1 = np.asarray(inputs["bias1"], dtype=np.float32)
    b2 = np.asarray(inputs["bias2"], dtype=np.float32)

    T = x.shape[0]
    E, IN, HID = w1.shape
    OUT = w2.shape[2]
    K1, M1 = IN // P, HID // P
    K2, M2 = HID // P, OUT // P
    assert E == N_CORES
    bf = ml_dtypes.bfloat16

    order = np.argsort(idx, kind="stable")
    counts = np.bincount(idx, minlength=E)
    starts = np.zeros(E + 1, dtype=np.int64)
    starts[1:] = np.cumsum(counts)
    # Capacity rounds to 16 (not 128); tiles split C near-equally.
    C = max(-(-int(counts.max()) // 16) * 16, 16)
    widths = _tile_widths(C)

    xbf = x.astype(bf)
    in_maps = []
    for e in range(E):
        toks = order[starts[e]:starts[e + 1]]
        xp = np.zeros((C, IN), dtype=bf)
        if len(toks):
            xp[:len(toks)] = xbf[toks]
        # per-tile blocks: (n0+w, k*128+p) -> [p, k*W+w], concatenated
        blocks = []
        n0 = 0
        for W in widths:
            blk = xp[n0:n0 + W].reshape(W, K1, P).transpose(2, 1, 0)
            blocks.append(blk.reshape(P, K1 * W))
            n0 += W
        x_sw = np.ascontiguousarray(np.concatenate(blocks, axis=1))
        # (k*128+p, m*128+c) -> [p, m, k*128+c]
        w1_sw = np.ascontiguousarray(
            w1[e].astype(bf).reshape(K1, P, M1, P).transpose(1, 2, 0, 3)
        ).reshape(P, M1, K1 * P)
        w2_sw = np.ascontiguousarray(
            w2[e].astype(bf).reshape(K2, P, M2, P).transpose(1, 2, 0, 3)
        ).reshape(P, M2, K2 * P)
        in_maps.append({
            "xT": x_sw,
            "w1": w1_sw,
            "w2": w2_sw,
            # biases pre-transposed to [P, M]
            "b1": np.ascontiguousarray(b1[e].reshape(M1, P).T),
            "b2": np.ascontiguousarray(b2[e].reshape(M2, P).T),
        })
    meta = {"key": (C, IN, HID, OUT), "order": order, "starts": starts,
            "T": T, "OUT": OUT}
    return in_maps, meta


def scatter_output(inputs, yT_all, meta):
    """Scatter per-core yT [E, OUT, C] back to [T, 1, OUT] fp32."""
    order, starts = meta["order"], meta["starts"]
    out = np.empty((meta["T"], meta["OUT"]), dtype=np.float32)
    for e in range(N_CORES):
        toks = order[starts[e]:starts[e + 1]]
        if len(toks):
            out[toks] = yT_all[e][:, :len(toks)].T
    return out[:, None, :]


def kernel(**inputs):
    in_maps, meta = prepare_in_maps(inputs)
    nc = _get_kernel(*meta["key"])
    res = run_bass_kernel_spmd(nc, in_maps, core_ids=list(range(N_CORES)),
                               trace=False)
    yT_all = np.stack([res.results[e]["yT"] for e in range(N_CORES)])
    return scatter_output(inputs, yT_all, meta)



# revision 21
# speedup vs baseline: 1.2106x; 1.0364x over previous
"""MoE runtime-experts kernel for 8 Trainium2 NeuronCores.

Expert-parallel: core e holds expert e's weights. Host routes tokens by
expert id (argsort), pads each expert batch to a common capacity C, and
each core computes y = gelu(x @ W1 + b1) @ W2 + b2 for its batch as
dense matmuls in a transposed layout:

    L1: hT[hid, tok]  = W1[in, hid].T-contract  xT[in, tok]
    L2: yT[out, tok]  = W2[hid, out].T-contract hT[hid, tok]

Weights stay stationary on the PE (lhsT), tokens are the moving dim, so
activations flow through both layers without any on-device transpose.
Inputs/weights are cast to bf16 on host (PSUM accumulates fp32).

All streamed tensors (x, w1, w2, biases) are pre-swizzled on the host to
partition-major layouts so every DMA window is ONE contiguous run per
SBUF partition (128 descriptors). Naive [IN, HID]-style layouts cost
32*128 descriptors per window, and the Sync engine's DMA_DIRECT2D issue
blocks on descriptor generation (~4ns/desc) — the profile showed 17us(!)
single issue instructions starving both the DMA queues and the PE.
"""

import numpy as np
import ml_dtypes

import concourse.bass as bass
import concourse.mybir as mybir
import concourse.tile as tile
from concourse import bacc
from concourse.bass_utils import run_bass_kernel_spmd

P = 128
N_CORES = 8
BF16 = mybir.dt.bfloat16
F32 = mybir.dt.float32

_nc_cache = {}


def _tile_widths(C):
    """Near-equal tile widths (multiples of 4), as few tiles as SBUF
    allows (W<=376 keeps x/h/o pools under the ~209KB/partition cap,
    and W<=512 fits one PSUM bank per chain).

    Widths are multiples of 8 so every per-k row stride (W*2 bytes)
    stays 16B-aligned in SBUF: misaligned strides slow the moving-
    operand stream ~20% (measured: W=256/272 clean at W/2.4+~3ns;
    W=260 -> 130ns, W=348 -> 176ns, both ~1.2x the aligned rate).
    Wider tiles amortize the fixed ~3ns/MM, but W caps at 272: the
    PIPE=2 interleaved prefix (which halves the startup w1 bandwidth
    demand — PIPE=1 at W~350 measured 12.5us of early PE starvation)
    needs 3 h-buffers, and SBUF fits 131KB weights + 288*W of pools
    only for W<=272 (quanta of 16 keep strides 32B-aligned). W >= ~240
    keeps the ~97ns LDWEIGHTS execute time hidden under the stream.
    Widest tile goes first (lowest prefix w1 demand; smallest last
    tile shortens the drain tail)."""
    n_t = max(1, -(-C // 272))
    base, rem = divmod(C // 16, n_t)
    return [16 * (base + (1 if i < rem else 0)) for i in range(n_t)]


def _ramp(total, first=1, cap=None):
    """Window sizes [1, 1, 2, 4, ...] (in m-tiles) doubling up to `cap`,
    summing to total. Small leading windows let dependent compute start
    early; big trailing windows amortize per-DMA issue overhead."""
    out, w, off = [], first, 0
    while off < total:
        take = min(w, total - off)
        out.append(take)
        off += take
        if len(out) >= 2:
            w *= 2
            if cap is not None:
                w = min(w, cap)
    return out


def _build_kernel(C, IN, HID, OUT, psum_bufs=8, repeat=1,
                  PIPE=2, GROUP=True, SPLIT_Y=True):
    K1, M1 = IN // P, HID // P
    K2, M2 = HID // P, OUT // P
    assert C % 16 == 0
    widths = _tile_widths(C)
    n_t = len(widths)
    starts_ = [sum(widths[:i]) for i in range(n_t)]
    xoffs = [K1 * s for s in starts_]
    # PSUM chains packed per 512-f32 bank: per-tile group size/stride.
    WMAX = max(widths)

    def _grp(W):
        gn = min(4, 512 // W)
        return gn, 512 // gn

    nc = bacc.Bacc("TRN2", target_bir_lowering=False, debug=False,
                   num_devices=N_CORES)
    # Swizzled layouts: tile t of xT is a [K1*W_t]-wide block per
    # partition, element (p, k*W_t+w) = x[token start_t+w, feature
    # k*128+p]; element (p, m, k*128+c) of w1 is w1[k*128+p, m*128+c];
    # likewise w2. Biases are [P, M] with (p, m) = b[m*128+p].
    xT = nc.dram_tensor("xT", [P, K1 * C], BF16, kind="ExternalInput")
    w1 = nc.dram_tensor("w1", [P, M1, K1 * P], BF16, kind="ExternalInput")
    w2 = nc.dram_tensor("w2", [P, M2, K2 * P], BF16, kind="ExternalInput")
    b1 = nc.dram_tensor("b1", [P, M1], F32, kind="ExternalInput")
    b2 = nc.dram_tensor("b2", [P, M2], F32, kind="ExternalInput")
    yT = nc.dram_tensor("yT", [OUT, C], F32, kind="ExternalOutput")

    with tile.TileContext(nc) as tc:
        with (
            tc.tile_pool(name="weights", bufs=1) as wpool,
            tc.tile_pool(name="xbuf",
                         bufs=(n_t if repeat == 1 else PIPE + 1)) as xpool,
            tc.tile_pool(name="hbuf", bufs=PIPE + 1) as hpool,
            tc.tile_pool(name="obuf",
                         bufs=(1 if WMAX > 200 else 2)) as opool,
            tc.tile_pool(name="psum", bufs=psum_bufs, space="PSUM") as pspool,
        ):
            w1_sb = wpool.tile([P, M1, K1 * P], BF16)
            w2_sb = wpool.tile([P, M2, K2 * P], BF16)
            b1_sb = wpool.tile([P, M1], F32)
            b2_sb = wpool.tile([P, M2], F32)

            depth = min(PIPE, n_t)
            # Single-shot: preload ALL x tiles into dedicated buffers (x is
            # only 2.3MB total). The first `depth` tiles go in front of the
            # weights so the L1 prefix can start immediately; the rest are
            # queued behind the weights (needed much later). DMA queue
            # order is issue order, so this sequencing is what keeps the
            # in-order Tensor stream fed.
            def _dma_x(x_sb, it, eng=None):
                nw = K1 * widths[it]
                (eng or nc.sync).dma_start(
                    x_sb[:, :nw], xT.ap()[:, xoffs[it]:xoffs[it] + nw])

            # Startup is DMA-ramp bound: queues deliver only ~100-180GB/s
            # for the first ~5us and each queue is FIFO, so the pieces the
            # first chain needs are spread across the THREE engine HWDGE
            # queues (sync/SP, scalar/Activation, gpsimd — the only ones
            # allowed to initiate DMAs) and issued in parallel (each DMA
            # issue costs ~650ns of its engine's time; engines issue
            # concurrently). First chain (m=0, k=0..7) gates on w1[m0]
            # (sync, two k-halves) and x0 (k-quarters alternating
            # scalar/gpsimd); biases follow on gpsimd. The rest of w1
            # ramps on sync in need order; every window is one contiguous
            # run per partition (128 descriptors).
            # Startup rides ONLY the sync + scalar hardware DGE queues:
            # gpsimd's DMA path is software-DGE and ~10x slower per
            # transfer (measured ~30GB/s — a 128KB piece straggled
            # 4.5us), so it gets nothing latency-critical. During the
            # 8-core simultaneous startup the device HBM is saturated
            # (~100GB/s per queue early), so the PE start is gated by
            # total prerequisite bytes, not queue count: x0 and w1's
            # first m-window are split so the first chain's k=0 matmul
            # (which range-tracks only the leading halves) starts after
            # ~320KB, and the trailing x pieces ride the scalar queue in
            # parallel with the sync queue's w1 stream.
            x_tiles = {}
            if repeat == 1:
                for it in range(n_t):
                    x_tiles[it] = xpool.tile([P, K1 * WMAX], BF16,
                                             tag="x", name="x_sb")
                hw = K1 * widths[0] // 2
                nc.sync.dma_start(x_tiles[0][:, :hw], xT.ap()[:, :hw])
            hm = K1 * P // 2
            nc.sync.dma_start(w1_sb[:, :1, :hm], w1.ap()[:, :1, :hm])
            if repeat == 1:
                nc.scalar.dma_start(x_tiles[0][:, hw:K1 * widths[0]],
                                    xT.ap()[:, hw:K1 * widths[0]])
            nc.sync.dma_start(w1_sb[:, :1, hm:], w1.ap()[:, :1, hm:])
            nc.sync.dma_start(b1_sb[:], b1.ap())
            if repeat == 1:
                for it in range(1, depth):
                    _dma_x(x_tiles[it], it, eng=nc.scalar)
            nc.sync.dma_start(b2_sb[:], b2.ap())
            off = 1
            for w in _ramp(M1 - 1, first=1, cap=4):
                nc.sync.dma_start(w1_sb[:, off:off + w],
                                  w1.ap()[:, off:off + w])
                off += w
            if repeat == 1:
                for it in range(depth, n_t):
                    _dma_x(x_tiles[it], it)
            off = 0
            for w in _ramp(M2, cap=2):
                nc.sync.dma_start(w2_sb[:, off:off + w],
                                  w2.ap()[:, off:off + w])
                off += w

            def l1_phase(it):
                x_sb = _get_x(it)
                W = widths[it]
                h_sb = hpool.tile([P, M1, WMAX], BF16, tag="h",
                                  name="h_sb")
                GN, SW = _grp(W)
                # Pack GN accumulation chains into one PSUM bank so the
                # slot-WAR sem wait is paid once per GN chains.
                for mg in range(0, M1, GN):
                    grp = range(mg, min(mg + GN, M1))
                    ps = pspool.tile([P, 512], F32, tag="ps", name="ps")
                    for mi, m in enumerate(grp):
                        for k in range(K1):
                            nc.tensor.matmul(
                                ps[:, mi * SW:mi * SW + W],
                                w1_sb[:, m, bass.ts(k, P)],
                                x_sb[:, k * W:(k + 1) * W],
                                start=(k == 0),
                                stop=(k == K1 - 1),
                            )
                    for mi, m in enumerate(grp):
                        nc.scalar.activation(
                            h_sb[:, m, :W],
                            ps[:, mi * SW:mi * SW + W],
                            mybir.ActivationFunctionType.Gelu,
                            bias=b1_sb[:, m:m + 1],
                        )
                return h_sb

            yTr = yT.ap().rearrange("(m p) c -> p m c", p=P)

            def l2_phase(it, h_sb, is_last=False):
                n0, W = starts_[it], widths[it]
                o_sb = opool.tile([P, M2, WMAX], F32, tag="o",
                                  name="o_sb")
                GN, SW = _grp(W)
                # Tail trim: on the last tile, each m-column's out-DMA is
                # issued right after its bias-add, alternating the two
                # hardware DGE queues (gpsimd's software DGE measured
                # ~10x slower — its transfers straggle past the stream
                # end), so the post-last-matmul drain is one m-column's
                # bias+issue+transfer instead of 8 transfers serialized
                # on the sync queue (~3us on the baseline trace).
                oeng = (nc.sync, nc.scalar)
                for mg in range(0, M2, GN):
                    grp = range(mg, min(mg + GN, M2))
                    ps = pspool.tile([P, 512], F32, tag="ps", name="ps")
                    for mi, m in enumerate(grp):
                        for k in range(K2):
                            nc.tensor.matmul(
                                ps[:, mi * SW:mi * SW + W],
                                w2_sb[:, m, bass.ts(k, P)],
                                h_sb[:, k, :W],
                                start=(k == 0),
                                stop=(k == K2 - 1),
                            )
                    for mi, m in enumerate(grp):
                        nc.vector.tensor_tensor(
                            o_sb[:, m, :W],
                            ps[:, mi * SW:mi * SW + W],
                            b2_sb[:, m:m + 1].to_broadcast((P, W)),
                            mybir.AluOpType.add,
                        )
                    if SPLIT_Y and is_last:
                        for mi, m in enumerate(grp):
                            oeng[m % 2].dma_start(yTr[:, m, n0:n0 + W],
                                                  o_sb[:, m, :W])
                if not (SPLIT_Y and is_last):
                    nc.sync.dma_start(yTr[:, :, n0:n0 + W], o_sb[:, :, :W])

            def _get_x(it):
                if it in x_tiles:
                    return x_tiles[it]
                x_sb = xpool.tile([P, K1 * WMAX], BF16, tag="x",
                                  name="x_sb")
                _dma_x(x_sb, it)
                return x_sb

            def l1_prefix(depth, hs):
                # m-interleaved L1 over the first `depth` tiles: one tile's
                # chains consume w1 m-columns at ~560GB/s, faster than the
                # ~430GB/s the DMA delivers w1 at startup. Spreading each
                # m-window over `depth` chains keeps the PE behind the DMA
                # so the weight load is fully hidden.
                xs = {it: _get_x(it) for it in range(depth)}
                for it in range(depth):
                    hs[it] = hpool.tile([P, M1, WMAX], BF16, tag="h",
                                        name="h_sb")
                GN, SW = _grp(max(widths[it] for it in range(depth)))
                chains = [(m, it) for m in range(M1) for it in range(depth)]
                for g in range(0, len(chains), GN):
                    grp = chains[g:g + GN]
                    ps = pspool.tile([P, 512], F32, tag="ps", name="ps")
                    for mi, (m, it) in enumerate(grp):
                        W = widths[it]
                        for k in range(K1):
                            nc.tensor.matmul(
                                ps[:, mi * SW:mi * SW + W],
                                w1_sb[:, m, bass.ts(k, P)],
                                xs[it][:, k * W:(k + 1) * W],
                                start=(k == 0),
                                stop=(k == K1 - 1),
                            )
                    for mi, (m, it) in enumerate(grp):
                        nc.scalar.activation(
                            hs[it][:, m, :widths[it]],
                            ps[:, mi * SW:mi * SW + widths[it]],
                            mybir.ActivationFunctionType.Gelu,
                            bias=b1_sb[:, m:m + 1],
                        )

            def body():
                # Software pipeline: L1 runs PIPE tiles ahead of L2 so the
                # w2 weight DMA tail hides behind L1 compute at startup.
                hs = {}
                l1_prefix(depth, hs)
                for j in range(n_t):
                    if j + depth < n_t:
                        hs[j + depth] = l1_phase(j + depth)
                    l2_phase(j, hs.pop(j), is_last=(j == n_t - 1))

            if repeat == 1:
                body()
            else:
                with tc.For_i(0, repeat, 1, name="rep"):
                    body()
    nc.compile()
    return nc


def _get_kernel(C, IN, HID, OUT):
    key = (C, IN, HID, OUT)
    if key not in _nc_cache:
        _nc_cache[key] = _build_kernel(C, IN, HID, OUT)
    return _nc_cache[key]


def prepare_in_maps(inputs):
    """Host-side routing: sort tokens by expert, pad to capacity C,
    build per-core swizzled input maps. Returns (in_maps, meta)."""
    x = np.ascontiguousarray(np.asarray(inputs["x"], dtype=np.float32))
    idx = np.asarray(inputs["indices_s"]).astype(np.int64)
    w1 = np.asarray(inputs["weight1"], dtype=np.float32)
    w2 = np.asarray(inputs["weight2"], dtype=np.float32)
    b1 = np.asarray(inputs["bias1"], dtype=np.float32)
    b2 = np.asarray(inputs["bias2"], dtype=np.float32)

    T = x.shape[0]
    E, IN, HID = w1.shape
    OUT = w2.shape[2]
    K1, M1 = IN // P, HID // P
    K2, M2 = HID // P, OUT // P
    assert E == N_CORES
    bf = ml_dtypes.bfloat16

    order = np.argsort(idx, kind="stable")
    counts = np.bincount(idx, minlength=E)
    starts = np.zeros(E + 1, dtype=np.int64)
    starts[1:] = np.cumsum(counts)
    # Capacity rounds to 16 (not 128); tiles split C near-equally.
    C = max(-(-int(counts.max()) // 16) * 16, 16)
    widths = _tile_widths(C)

    xbf = x.astype(bf)
    in_maps = []
    for e in range(E):
        toks = order[starts[e]:starts[e + 1]]
        xp = np.zeros((C, IN), dtype=bf)
        if len(toks):
            xp[:len(toks)] = xbf[toks]
        # per-tile blocks: (n0+w, k*128+p) -> [p, k*W+w], concatenated
        blocks = []
        n0 = 0
        for W in widths:
            blk = xp[n0:n0 + W].reshape(W, K1, P).transpose(2, 1, 0)
            blocks.append(blk.reshape(P, K1 * W))
            n0 += W
        x_sw = np.ascontiguousarray(np.concatenate(blocks, axis=1))
        # (k*128+p, m*128+c) -> [p, m, k*128+c]
        w1_sw = np.ascontiguousarray(
            w1[e].astype(bf).reshape(K1, P, M1, P).transpose(1, 2, 0, 3)
        ).reshape(P, M1, K1 * P)
        w2_sw = np.ascontiguousarray(
            w2[e].astype(bf).reshape(K2, P, M2, P).transpose(1, 2, 0, 3)
        ).reshape(P, M2, K2 * P)
        in_maps.append({
            "xT": x_sw,
            "w1": w1_sw,
            "w2": w2_sw,
            # biases pre-transposed to [P, M]
            "b1": np.ascontiguousarray(b1[e].reshape(M1, P).T),
            "b2": np.ascontiguousarray(b2[e].reshape(M2, P).T),
        })
    meta = {"key": (C, IN, HID, OUT), "order": order, "starts": starts,
            "T": T, "OUT": OUT}
    return in_maps, meta


def scatter_output(inputs, yT_all, meta):
    """Scatter per-core yT [E, OUT, C] back to [T, 1, OUT] fp32."""
    order, starts = meta["order"], meta["starts"]
    out = np.empty((meta["T"], meta["OUT"]), dtype=np.float32)
    for e in range(N_CORES):
        toks = order[starts[e]:starts[e + 1]]
        if len(toks):
            out[toks] = yT_all[e][:, :len(toks)].T
    return out[:, None, :]


def kernel(**inputs):
    in_maps, meta = prepare_in_maps(inputs)
    nc = _get_kernel(*meta["key"])
    res = run_bass_kernel_spmd(nc, in_maps, core_ids=list(range(N_CORES)),
                               trace=False)
    yT_all = np.stack([res.results[e]["yT"] for e in range(N_CORES)])
    return scatter_output(inputs, yT_all, meta)

